# revision 9
# baseline (speedup 1.0000x reference)
# ListFold loss (exponential transform, beta=1) on 8 Trainium2 NeuronCores.
#
# Math: with sp = pred sorted by target descending, the reference computes
#   loss = sum_i log(den_i) - (sp[i] - sp[n-1-i]),  i in [0, n/2)
#   den_i = (cp[n-i]-cp[i]) * (cm[n-i]-cm[i]) - (n-2i)
# where cp/cm are prefix sums of exp(+-sp). Re-indexing from the middle
# outward with t = n/2-1-i, u[t] = sp[n/2-1-t], v[t] = sp[n/2+t]:
#   s_plus(t)  = cumsum_incl(exp(u)+exp(v))[t]      (= cp[n-i]-cp[i])
#   s_minus(t) = cumsum_incl(exp(-u)+exp(-v))[t]
#   loss = sum_t log(s_plus*s_minus - (2t+2)) - (u[t]-v[t])
# Window sums are exact (no differencing of large prefix sums) and, by
# Cauchy-Schwarz, s_plus*s_minus >= L^2, so den >= L(L-1) > 0: the bf16
# element streams below cannot produce a negative log argument.
# The log_num part enters only through a global sum: sum_t (u-v) is
# computed on the host during unshard (two block sums, no device work).
#
# Sharding: the pair index t is split into 8 contiguous blocks, one per
# core, laid out [128 partitions x 4096] partition-major. Each core scans
# its block chunk-by-chunk along the free axis (tensor_tensor_scan with
# per-chunk initial=0; the chunk carry is folded into the phase-B
# per-partition scalars), resolves the partition-axis carry with a
# strict-triangular matmul, and the cross-core carry with a direct
# peer-DMA all-gather of per-core totals (remote_dma_broadcast, one slot
# per XOR offset) instead of a collective_compute AllReduce - the mesh
# AllReduce costs ~38us in trigger-to-done latency, the peer exchange
# ~2us. Each payload carries (total_plus, total_minus, host_rank), so the
# prefix mask is computed on-device with is_lt and is immune to the
# logical-to-physical core permutation. Block totals come from ACT-side
# accum_out on the Exp activations, so the exchange trigger never waits
# on the vector queue (which is busy with scans). Elementwise streams are
# bf16 for 2x DVE throughput; scan state is fp32 internally per the ISA,
# and all f32 carries/accumulators keep the result well inside the
# tolerance. Per-core partial losses are summed on the host (the unshard
# step). The argsort itself is int bookkeeping done on the host while
# sharding (XLA cannot sort on trn2 at all).

import numpy as np

N = 8388608
H = N // 2          # pairs
NCORES = 8
B = H // NCORES     # pairs per core
P = 128
C = B // P          # 4096 free-dim columns
F = 1024            # phase chunk width
NCHUNK = C // F

_CACHE = {}


def _build_nc():
    import concourse.bacc as bacc
    import concourse.mybir as mybir
    import concourse.tile as tile

    dt = mybir.dt
    f32 = dt.float32
    bf16 = dt.bfloat16
    Alu = mybir.AluOpType
    Act = mybir.ActivationFunctionType

    nc = bacc.Bacc("TRN2", target_bir_lowering=False, debug=False,
                   num_devices=NCORES)

    u_in = nc.dram_tensor("u_in", [P, C], f32, kind="ExternalInput").ap()
    v_in = nc.dram_tensor("v_in", [P, C], f32, kind="ExternalInput").ap()
    strict = nc.dram_tensor("strict", [P, P], f32, kind="ExternalInput").ap()
    ones_col = nc.dram_tensor("ones_col", [P, 1], f32, kind="ExternalInput").ap()
    rank2 = nc.dram_tensor("rank2", [P, 2], f32, kind="ExternalInput").ap()
    neg_lbase = nc.dram_tensor("neg_lbase", [P, 1], f32, kind="ExternalInput").ap()
    out_part = nc.dram_tensor("partial", [1, 1], f32, kind="ExternalOutput").ap()

    rsem = nc.alloc_semaphore("peer_rsem")
    lsem = nc.alloc_semaphore("peer_lsem")

    with tile.TileContext(nc) as tc:
        with (
            tc.tile_pool(name="const", bufs=1) as constp,
            tc.tile_pool(name="big", bufs=1) as bigp,
            tc.tile_pool(name="work", bufs=2) as workp,
            tc.tile_pool(name="small", bufs=1) as smallp,
            tc.tile_pool(name="acc", bufs=1) as accp,
            tc.tile_pool(name="psum", bufs=1, space="PSUM") as psump,
        ):
            strict_t = constp.tile([P, P], f32, tag="strict")
            ones_col_t = constp.tile([P, 1], f32, tag="ones_col")
            neg_lbase_t = constp.tile([P, 1], f32, tag="neg_lbase")
            allones_t = constp.tile([P, P], f32, tag="allones")

            # L(t_local) = 2*(p*C + c) + 2; bf16 rounding of L is harmless:
            # den >= L(L-1), so the relative den error from rounding L is
            # <= 2^-9 * L / (L-1).
            iota_t = bigp.tile([P, C], bf16, tag="iota")
            nc.gpsimd.iota(iota_t[:], pattern=[[2, C]], base=2,
                           channel_multiplier=2 * C,
                           allow_small_or_imprecise_dtypes=True)

            wp_t = bigp.tile([P, C], bf16, tag="wp")   # exp(u)+exp(v)
            wm_t = bigp.tile([P, C], bf16, tag="wm")   # exp(-u)+exp(-v)
            sp_t = bigp.tile([P, C], bf16, tag="sp")   # per-chunk scan of wp
            sm_t = bigp.tile([P, C], bf16, tag="sm")   # per-chunk scan of wm
            x1_t = bigp.tile([P, C], bf16, tag="x1")   # sp*sm - iota

            awp = accp.tile([P, 2 * NCHUNK], f32, tag="awp")  # row sums e^u,e^v
            awm = accp.tile([P, 2 * NCHUNK], f32, tag="awm")
            aln = accp.tile([P, NCHUNK], f32, tag="aln")      # row sums of ln

            nc.vector.memset(allones_t[:], 1.0)

            # ---- phase A: exps (with ACT-side row totals), pair sums,
            # per-chunk scans, carry-independent x1 ----
            for c in range(NCHUNK):
                cs = slice(c * F, (c + 1) * F)
                u_t = workp.tile([P, F], f32, tag="u")
                v_t = workp.tile([P, F], f32, tag="v")
                nc.sync.dma_start(u_t[:], u_in[:, cs])
                nc.sync.dma_start(v_t[:], v_in[:, cs])

                eu = workp.tile([P, F], bf16, tag="eu")
                ev = workp.tile([P, F], bf16, tag="ev")
                emu = workp.tile([P, F], bf16, tag="emu")
                emv = workp.tile([P, F], bf16, tag="emv")
                # accum_out rides the activations: block totals never wait
                # on the (busy) vector queue.
                nc.scalar.activation(eu[:], u_t[:], Act.Exp,
                                     accum_out=awp[:, 2 * c:2 * c + 1])
                nc.scalar.activation(ev[:], v_t[:], Act.Exp,
                                     accum_out=awp[:, 2 * c + 1:2 * c + 2])
                nc.scalar.activation(emu[:], u_t[:], Act.Exp, scale=-1.0,
                                     accum_out=awm[:, 2 * c:2 * c + 1])
                nc.scalar.activation(emv[:], v_t[:], Act.Exp, scale=-1.0,
                                     accum_out=awm[:, 2 * c + 1:2 * c + 2])

                nc.vector.tensor_add(wp_t[:, cs], eu[:], ev[:])
                nc.vector.tensor_add(wm_t[:, cs], emu[:], emv[:])

                # chunk scans chained through the previous chunk's last
                # column (fp32 state within a chunk; the bf16 rounding at
                # chunk boundaries is the same order as the element noise)
                ip = 0.0 if c == 0 else sp_t[:, c * F - 1:c * F]
                im = 0.0 if c == 0 else sm_t[:, c * F - 1:c * F]
                nc.vector.tensor_tensor_scan(
                    sp_t[:, cs], wp_t[:, cs], wp_t[:, cs], ip,
                    Alu.add, Alu.bypass)
                nc.vector.tensor_tensor_scan(
                    sm_t[:, cs], wm_t[:, cs], wm_t[:, cs], im,
                    Alu.add, Alu.bypass)

                prod = workp.tile([P, F], bf16, tag="prod")
                nc.vector.tensor_mul(prod[:], sp_t[:, cs], sm_t[:, cs])
                nc.vector.tensor_sub(x1_t[:, cs], prod[:], iota_t[:, cs])

            # consts are needed from the carry stage on - issue their DMAs
            # after the phase-A loads so chunk 0 starts sooner
            payload = smallp.tile([P, 4], f32, tag="payload")
            nc.sync.dma_start(strict_t[:], strict)
            nc.sync.dma_start(ones_col_t[:], ones_col)
            nc.sync.dma_start(neg_lbase_t[:], neg_lbase)
            nc.sync.dma_start(payload[:, 2:4], rank2)

            # ---- block totals: [P,2] row totals and their partition sums ----
            rt2 = smallp.tile([P, 2], f32, tag="rt2")
            nc.vector.tensor_reduce(rt2[:, 0:1], awp[:],
                                    axis=mybir.AxisListType.X, op=Alu.add)
            nc.vector.tensor_reduce(rt2[:, 1:2], awm[:],
                                    axis=mybir.AxisListType.X, op=Alu.add)

            tot_ps = psump.tile([P, 2], f32, tag="tot")
            nc.tensor.matmul(tot_ps[:], allones_t[:], rt2[:],
                             start=True, stop=True)
            nc.scalar.copy(payload[:, 0:2], tot_ps[:])

            # ---- carry exchange: peer-DMA all-gather of (totp, totm, rank).
            # Slot m on every core receives from the unique peer at XOR
            # offset m; slot 0 is the local copy. 7 broadcasts x 2 lanes
            # increment rsem to 14 when all peer totals have landed. ----
            slots = smallp.tile([P, 4 * NCORES], f32, tag="slots")
            nc.vector.tensor_copy(slots[:, 0:4], payload[:])
            for m in range(1, NCORES):
                rdests = [(0, m) if j == m else None for j in range(NCORES)]
                nc.gpsimd.remote_dma_broadcast(
                    out_ap=slots[:, 4 * m:4 * m + 4],
                    in_ap=payload[:],
                    remote_sem=rsem,
                    local_sem=lsem,
                    rdests=rdests,
                )
            nc.gpsimd.trigger_dma(count=None)

            # local strict-prefix part of the carry: ready pre-exchange
            loc_ps = psump.tile([P, 2], f32, tag="loc")
            nc.tensor.matmul(loc_ps[:], strict_t[:], rt2[:],
                             start=True, stop=True)

            # warm the Ln table while ACT is idle (input: any ready f32)
            lnwarm = smallp.tile([P, 1], f32, tag="lnwarm")
            nc.scalar.activation(lnwarm[:], rt2[:, 0:1], Act.Ln)

            # ---- post-exchange: mask peers with rank < mine, reduce ----
            totp_v = slots[:, 0:4 * NCORES:4]   # [P,8] stride-4 views
            totm_v = slots[:, 1:4 * NCORES:4]
            rank_v = slots[:, 2:4 * NCORES:4]

            mask8 = smallp.tile([P, NCORES], f32, tag="mask8")
            mscp = smallp.tile([P, NCORES], f32, tag="mscp")
            mscm = smallp.tile([P, NCORES], f32, tag="mscm")
            cb2 = smallp.tile([P, 2], f32, tag="cb2")
            # The rsem>=14 gate is inserted post-scheduling (the tile
            # scheduling sim cannot model remote arrivals and would
            # deadlock); here only the data deps order these after the
            # local slot writes. tensor_tensor_reduce is avoided - it
            # crashes this runtime (probe-verified); mul+reduce instead.
            readers = []
            readers.append(nc.vector.tensor_scalar(
                out=mask8[:], in0=rank_v, scalar1=payload[:, 2:3],
                scalar2=None, op0=Alu.is_lt,
            ).ins)
            readers.append(nc.vector.tensor_mul(mscp[:], mask8[:], totp_v).ins)
            readers.append(nc.vector.tensor_mul(mscm[:], mask8[:], totm_v).ins)
            nc.vector.tensor_reduce(cb2[:, 0:1], mscp[:],
                                    axis=mybir.AxisListType.X, op=Alu.add)
            nc.vector.tensor_reduce(cb2[:, 1:2], mscm[:],
                                    axis=mybir.AxisListType.X, op=Alu.add)

            carry_sb = smallp.tile([P, 2], f32, tag="carry_sb")
            nc.vector.tensor_add(carry_sb[:], cb2[:], loc_ps[:])

            # bias = Cp*Cm - 2kB (per-partition scalars for the Ln)
            cpcm = smallp.tile([P, 1], f32, tag="cpcm")
            nc.vector.tensor_scalar(
                out=cpcm[:], in0=carry_sb[:, 0:1], scalar1=carry_sb[:, 1:2],
                scalar2=None, op0=Alu.mult)
            bias_t = smallp.tile([P, 1], f32, tag="bias_t")
            nc.vector.tensor_add(bias_t[:], cpcm[:], neg_lbase_t[:])

            # ---- phase B: den = x1 + Cp*sm + Cm*sp + (CpCm - 2kB), log ----
            for c in range(NCHUNK):
                cs = slice(c * F, (c + 1) * F)
                t1 = workp.tile([P, F], bf16, tag="t1")
                nc.vector.scalar_tensor_tensor(
                    out=t1[:], in0=sm_t[:, cs], scalar=carry_sb[:, 0:1],
                    in1=x1_t[:, cs], op0=Alu.mult, op1=Alu.add)
                t2 = workp.tile([P, F], bf16, tag="t2")
                nc.vector.scalar_tensor_tensor(
                    out=t2[:], in0=sp_t[:, cs], scalar=carry_sb[:, 1:2],
                    in1=t1[:], op0=Alu.mult, op1=Alu.add)
                ln_o = workp.tile([P, F], bf16, tag="lnscratch")
                nc.scalar.activation(ln_o[:], t2[:], Act.Ln,
                                     bias=bias_t[:],
                                     accum_out=aln[:, c:c + 1])

            rll = smallp.tile([P, 1], f32, tag="rll")
            nc.vector.tensor_reduce(rll[:], aln[:], axis=mybir.AxisListType.X,
                                    op=Alu.add)

            part_ps = psump.tile([1, 1], f32, tag="part")
            nc.tensor.matmul(part_ps[:], ones_col_t[:], rll[:],
                             start=True, stop=True)
            part_sb = smallp.tile([1, 1], f32, tag="part_sb")
            nc.scalar.copy(part_sb[:], part_ps[:])
            nc.sync.dma_start(out_part, part_sb[:])

    # ---- post-scheduling: arrival gate + sem restore -----------------------
    # Insert a DVE event-semaphore wait (rsem >= 14) immediately before the
    # earliest scheduled slot reader. The tile scheduler has already ordered
    # the readers after the local slot writes; this wait adds the cross-core
    # arrival condition the scheduler cannot see.
    nwait = 2 * (NCORES - 1)
    gate = nc.vector.wait_ge(rsem, nwait).ins
    reader_names = {i.name for i in readers}
    placed = False
    for bb in nc.cur_f.blocks:
        idxs = [i for i, ins in enumerate(bb.instructions)
                if ins.name in reader_names]
        if idxs:
            cur = nc.cur_f.blocks[-1]
            if gate in cur.instructions:
                cur.instructions.remove(gate)
            bb.instructions.insert(min(idxs), gate)
            placed = True
            break
    assert placed, "slot readers not found in scheduled blocks"

    # rsem is left at 14 on kernel exit; the runtime zeroes semaphores per
    # NEFF execution (the fixed DMA-completion thresholds all over this
    # kernel rely on the same behavior).

    nc.compile()
    return nc


def _get_nc():
    if "nc" not in _CACHE:
        _CACHE["nc"] = _build_nc()
    return _CACHE["nc"]


def _make_in_maps(pred, target):
    pred = np.ascontiguousarray(np.asarray(pred, dtype=np.float32))
    target = np.ascontiguousarray(np.asarray(target, dtype=np.float32))
    assert pred.shape == (N,) and target.shape == (N,)

    order = np.argsort(-target, kind="stable")  # matches jnp stable argsort
    sp = pred[order]
    u = sp[H - 1:: -1]  # sp[H-1-t]
    v = sp[H:]          # sp[H+t]

    strict = np.triu(np.ones((P, P), np.float32), 1)  # [q,p]=1 iff q<p
    ones_col = np.ones((P, 1), np.float32)

    in_maps = []
    for k in range(NCORES):
        rank2 = np.zeros((P, 2), np.float32)
        rank2[:, 0] = np.float32(k)
        in_maps.append({
            "u_in": np.ascontiguousarray(u[k * B:(k + 1) * B].reshape(P, C)),
            "v_in": np.ascontiguousarray(v[k * B:(k + 1) * B].reshape(P, C)),
            "strict": strict,
            "ones_col": ones_col,
            "rank2": rank2,
            "neg_lbase": np.full((P, 1), -2.0 * k * B, np.float32),
        })
    return in_maps, u, v


def _run(in_maps, trace=False):
    from concourse import bass_utils
    return bass_utils.run_bass_kernel_spmd(
        _get_nc(), in_maps, list(range(NCORES)), trace=trace
    )


def _finish(res, u, v):
    partials = [r["partial"].reshape(()) for r in res.results]
    lnsum = np.sum(np.asarray(partials, dtype=np.float64))
    log_num = np.sum(u.astype(np.float64)) - np.sum(v.astype(np.float64))
    loss = np.float32(lnsum - log_num)
    return np.asarray(loss, dtype=np.float32).reshape(())


def kernel(pred, target):
    in_maps, u, v = _make_in_maps(pred, target)
    res = _run(in_maps)
    return _finish(res, u, v)


def kernel_traced(pred, target):
    in_maps, u, v = _make_in_maps(pred, target)
    res = _run(in_maps, trace=True)
    return _finish(res, u, v), res


# revision 12
# speedup vs baseline: 62.9540x; 62.9540x over previous
# ListFold loss (exponential transform, beta=1) on 8 Trainium2 NeuronCores.
#
# Math: with sp = pred sorted by target descending, the reference computes
#   loss = sum_i log(den_i) - (sp[i] - sp[n-1-i]),  i in [0, n/2)
#   den_i = (cp[n-i]-cp[i]) * (cm[n-i]-cm[i]) - (n-2i)
# where cp/cm are prefix sums of exp(+-sp). Re-indexing from the middle
# outward with t = n/2-1-i, u[t] = sp[n/2-1-t], v[t] = sp[n/2+t]:
#   s_plus(t)  = cumsum_incl(exp(u)+exp(v))[t]      (= cp[n-i]-cp[i])
#   s_minus(t) = cumsum_incl(exp(-u)+exp(-v))[t]
#   loss = sum_t log(s_plus*s_minus - (2t+2)) - (u[t]-v[t])
# Window sums are exact (no differencing of large prefix sums) and, by
# Cauchy-Schwarz, s_plus*s_minus >= L^2, so den >= L(L-1) > 0: the bf16
# element streams below cannot produce a negative log argument. The
# log_num part enters only through a global sum: sum_t (u-v) is computed
# on the host during unshard (two block sums, no device work).
#
# Sharding: the pair index t is split into 8 contiguous blocks, one per
# core, laid out [128 partitions x 4096] partition-major. Each core scans
# its block chunk-by-chunk along the free axis (tensor_tensor_scan chained
# through the previous chunk's last column), resolves the partition-axis
# carry with a strict-triangular matmul, and the cross-core carry with one
# [8,2] AllReduce of per-block totals (scan-style carry exchange). Block
# totals ride accum_out on the Exp activations, so the AllReduce trigger
# path never waits on the vector queue (which is busy with scans) - on
# the slowest-starting core every cycle of trigger delay lands directly
# on the collective's completion time. Elementwise pair/product streams
# are bf16 (2x DVE for tensor_tensor; scan state is fp32 internally per
# the ISA). Phase B applies the carry on the otherwise-idle PE as two
# diag(carry) matmuls accumulated in PSUM, leaving one DVE add + one Ln
# per chunk after the collective. Per-core partial losses are summed on
# the host (the unshard step). The argsort is int bookkeeping done on the
# host while sharding (XLA cannot sort on trn2 at all).
#
# (A direct peer remote_dma_broadcast exchange instead of the AllReduce
# was probe-verified functional but takes 6-8 ms under this runtime's
# host-emulated SWDGE path, so the hardware collective stays.)

import numpy as np

N = 8388608
H = N // 2          # pairs
NCORES = 8
B = H // NCORES     # pairs per core
P = 128
C = B // P          # 4096 free-dim columns
F = 1024            # phase chunk width
NCHUNK = C // F

_CACHE = {}


def _build_nc():
    import concourse.bacc as bacc
    import concourse.mybir as mybir
    import concourse.tile as tile

    dt = mybir.dt
    f32 = dt.float32
    bf16 = dt.bfloat16
    Alu = mybir.AluOpType
    Act = mybir.ActivationFunctionType

    nc = bacc.Bacc("TRN2", target_bir_lowering=False, debug=False,
                   num_devices=NCORES)

    u_in = nc.dram_tensor("u_in", [P, C], f32, kind="ExternalInput").ap()
    v_in = nc.dram_tensor("v_in", [P, C], f32, kind="ExternalInput").ap()
    hotcol = nc.dram_tensor("hotcol", [P, NCORES], f32, kind="ExternalInput").ap()
    maskbc = nc.dram_tensor("maskbc", [NCORES, P], f32, kind="ExternalInput").ap()
    strict = nc.dram_tensor("strict", [P, P], f32, kind="ExternalInput").ap()
    ident = nc.dram_tensor("ident", [P, P], bf16, kind="ExternalInput").ap()
    ones_col = nc.dram_tensor("ones_col", [P, 1], f32, kind="ExternalInput").ap()
    neg_lbase = nc.dram_tensor("neg_lbase", [P, 1], f32, kind="ExternalInput").ap()
    out_part = nc.dram_tensor("partial", [1, 1], f32, kind="ExternalOutput").ap()

    with tile.TileContext(nc) as tc:
        with (
            tc.tile_pool(name="const", bufs=1) as constp,
            tc.tile_pool(name="big", bufs=1) as bigp,
            tc.tile_pool(name="work", bufs=2) as workp,
            tc.tile_pool(name="small", bufs=1) as smallp,
            tc.tile_pool(name="acc", bufs=1) as accp,
            tc.tile_pool(name="psum", bufs=1, space="PSUM") as psump,
            tc.tile_pool(name="psumB", bufs=2, space="PSUM") as psumbp,
            tc.tile_pool(name="dram", bufs=1, space="DRAM") as dramp,
        ):
            strict_t = constp.tile([P, P], f32, tag="strict")
            ident_t = constp.tile([P, P], bf16, tag="ident")
            hotcol_t = constp.tile([P, NCORES], f32, tag="hotcol")
            maskbc_t = constp.tile([NCORES, P], f32, tag="maskbc")
            ones_col_t = constp.tile([P, 1], f32, tag="ones_col")
            neg_lbase_t = constp.tile([P, 1], f32, tag="neg_lbase")

            # L(t_local) = 2*(p*C + c) + 2; bf16 rounding of L is harmless:
            # den >= L(L-1) makes the relative den error <= 2^-9 * L/(L-1).
            iota_t = bigp.tile([P, C], bf16, tag="iota")
            nc.gpsimd.iota(iota_t[:], pattern=[[2, C]], base=2,
                           channel_multiplier=2 * C,
                           allow_small_or_imprecise_dtypes=True)

            wp_t = bigp.tile([P, C], bf16, tag="wp")   # exp(u)+exp(v)
            wm_t = bigp.tile([P, C], bf16, tag="wm")   # exp(-u)+exp(-v)
            sp_t = bigp.tile([P, C], bf16, tag="sp")   # running scan of wp
            sm_t = bigp.tile([P, C], bf16, tag="sm")   # running scan of wm
            x1_t = bigp.tile([P, C], bf16, tag="x1")   # sp*sm - iota

            awp = accp.tile([P, 2 * NCHUNK], f32, tag="awp")  # row sums e^u,e^v
            awm = accp.tile([P, 2 * NCHUNK], f32, tag="awm")
            aln = accp.tile([P, NCHUNK], f32, tag="aln")      # row sums of ln

            # ---- phase A: exps (with ACT-side row totals), pair sums,
            # chained chunk scans, carry-independent x1 ----
            for c in range(NCHUNK):
                cs = slice(c * F, (c + 1) * F)
                u_t = workp.tile([P, F], f32, tag="u")
                v_t = workp.tile([P, F], f32, tag="v")
                nc.sync.dma_start(u_t[:], u_in[:, cs])
                nc.sync.dma_start(v_t[:], v_in[:, cs])

                eu = workp.tile([P, F], bf16, tag="eu")
                ev = workp.tile([P, F], bf16, tag="ev")
                emu = workp.tile([P, F], bf16, tag="emu")
                emv = workp.tile([P, F], bf16, tag="emv")
                # accum_out rides the activations: block totals never wait
                # on the (busy) vector queue.
                nc.scalar.activation(eu[:], u_t[:], Act.Exp,
                                     accum_out=awp[:, 2 * c:2 * c + 1])
                nc.scalar.activation(ev[:], v_t[:], Act.Exp,
                                     accum_out=awp[:, 2 * c + 1:2 * c + 2])
                nc.scalar.activation(emu[:], u_t[:], Act.Exp, scale=-1.0,
                                     accum_out=awm[:, 2 * c:2 * c + 1])
                nc.scalar.activation(emv[:], v_t[:], Act.Exp, scale=-1.0,
                                     accum_out=awm[:, 2 * c + 1:2 * c + 2])

                nc.vector.tensor_add(wp_t[:, cs], eu[:], ev[:])
                nc.vector.tensor_add(wm_t[:, cs], emu[:], emv[:])

                ip = 0.0 if c == 0 else sp_t[:, c * F - 1:c * F]
                im = 0.0 if c == 0 else sm_t[:, c * F - 1:c * F]
                nc.vector.tensor_tensor_scan(
                    sp_t[:, cs], wp_t[:, cs], wp_t[:, cs], ip,
                    Alu.add, Alu.bypass)
                nc.vector.tensor_tensor_scan(
                    sm_t[:, cs], wm_t[:, cs], wm_t[:, cs], im,
                    Alu.add, Alu.bypass)

                prod = workp.tile([P, F], bf16, tag="prod")
                nc.vector.tensor_mul(prod[:], sp_t[:, cs], sm_t[:, cs])
                nc.vector.tensor_sub(x1_t[:, cs], prod[:], iota_t[:, cs])

            # consts are needed from the carry stage on - issue their DMAs
            # after the phase-A loads so chunk 0 starts sooner
            nc.sync.dma_start(strict_t[:], strict)
            nc.sync.dma_start(ident_t[:], ident)
            nc.sync.dma_start(hotcol_t[:], hotcol)
            nc.sync.dma_start(maskbc_t[:], maskbc)
            nc.sync.dma_start(ones_col_t[:], ones_col)
            nc.sync.dma_start(neg_lbase_t[:], neg_lbase)

            # ---- block totals -> one matmul -> AllReduce trigger ----
            rt2 = smallp.tile([P, 2], f32, tag="rt2")
            nc.vector.tensor_reduce(rt2[:, 0:1], awp[:],
                                    axis=mybir.AxisListType.X, op=Alu.add)
            nc.vector.tensor_reduce(rt2[:, 1:2], awm[:],
                                    axis=mybir.AxisListType.X, op=Alu.add)

            contrib_ps = psump.tile([NCORES, 2], f32, tag="contrib")
            nc.tensor.matmul(contrib_ps[:], hotcol_t[:], rt2[:],
                             start=True, stop=True)
            contrib_sb = smallp.tile([NCORES, 2], f32, tag="contrib_sb")
            nc.scalar.copy(contrib_sb[:], contrib_ps[:])

            cc_in = dramp.tile([NCORES, 2], f32, tag="cc_in")
            cc_out = dramp.tile([NCORES, 2], f32, tag="cc_out")
            nc.sync.dma_start(cc_in[:], contrib_sb[:])
            nc.gpsimd.collective_compute(
                "AllReduce", Alu.add,
                replica_groups=[list(range(NCORES))],
                ins=[cc_in.opt()], outs=[cc_out.opt()])
            allt = smallp.tile([NCORES, 2], f32, tag="allt")
            nc.sync.dma_start(allt[:], cc_out[:])

            # carry = strict-local prefix + broadcast of earlier cores'
            # totals, both accumulated into one PSUM tile
            carry_ps = psump.tile([P, 2], f32, tag="carry")
            nc.tensor.matmul(carry_ps[:], strict_t[:], rt2[:],
                             start=True, stop=False)
            nc.tensor.matmul(carry_ps[:], maskbc_t[:], allt[:],
                             start=False, stop=True)
            carry_sb = smallp.tile([P, 2], f32, tag="carry_sb")
            nc.scalar.copy(carry_sb[:], carry_ps[:])

            # warm the Ln table while ACT is idle (input: any ready f32)
            lnwarm = smallp.tile([P, 1], f32, tag="lnwarm")
            nc.scalar.activation(lnwarm[:], rt2[:, 0:1], Act.Ln)

            # diag(Cp), diag(Cm) as bf16 lhsT for the phase-B PE matmuls
            dcp = smallp.tile([P, P], bf16, tag="dcp")
            dcm = smallp.tile([P, P], bf16, tag="dcm")
            nc.vector.tensor_scalar(
                out=dcp[:], in0=ident_t[:], scalar1=carry_sb[:, 0:1],
                scalar2=None, op0=Alu.mult)
            nc.vector.tensor_scalar(
                out=dcm[:], in0=ident_t[:], scalar1=carry_sb[:, 1:2],
                scalar2=None, op0=Alu.mult)

            # bias = Cp*Cm - 2kB (per-partition scalar for the Ln)
            cpcm = smallp.tile([P, 1], f32, tag="cpcm")
            nc.vector.tensor_scalar(
                out=cpcm[:], in0=carry_sb[:, 0:1], scalar1=carry_sb[:, 1:2],
                scalar2=None, op0=Alu.mult)
            bias_t = smallp.tile([P, 1], f32, tag="bias_t")
            nc.vector.tensor_add(bias_t[:], cpcm[:], neg_lbase_t[:])

            # ---- phase B: den = x1 + Cp*sm + Cm*sp + (CpCm - 2kB), log.
            # The two carry products run on the PE (PSUM-accumulated);
            # DVE only adds x1, ACT takes the log. ----
            HB = 512  # one PSUM bank of f32 per matmul output
            for c in range(NCHUNK):
                cs = slice(c * F, (c + 1) * F)
                t2 = workp.tile([P, F], bf16, tag="t2")
                for h in range(F // HB):
                    hs = slice(c * F + h * HB, c * F + (h + 1) * HB)
                    ps_h = psumbp.tile([P, HB], f32, tag="psB")
                    nc.tensor.matmul(ps_h[:], dcp[:], sm_t[:, hs],
                                     start=True, stop=False)
                    nc.tensor.matmul(ps_h[:], dcm[:], sp_t[:, hs],
                                     start=False, stop=True)
                    nc.vector.tensor_add(t2[:, h * HB:(h + 1) * HB],
                                         x1_t[:, hs], ps_h[:])
                ln_o = workp.tile([P, F], bf16, tag="lnscratch")
                nc.scalar.activation(ln_o[:], t2[:], Act.Ln,
                                     bias=bias_t[:],
                                     accum_out=aln[:, c:c + 1])

            rll = smallp.tile([P, 1], f32, tag="rll")
            nc.vector.tensor_reduce(rll[:], aln[:], axis=mybir.AxisListType.X,
                                    op=Alu.add)

            part_ps = psump.tile([1, 1], f32, tag="part")
            nc.tensor.matmul(part_ps[:], ones_col_t[:], rll[:],
                             start=True, stop=True)
            part_sb = smallp.tile([1, 1], f32, tag="part_sb")
            nc.scalar.copy(part_sb[:], part_ps[:])
            nc.sync.dma_start(out_part, part_sb[:])

    nc.compile()
    return nc


def _get_nc():
    if "nc" not in _CACHE:
        _CACHE["nc"] = _build_nc()
    return _CACHE["nc"]


def _make_in_maps(pred, target):
    pred = np.ascontiguousarray(np.asarray(pred, dtype=np.float32))
    target = np.ascontiguousarray(np.asarray(target, dtype=np.float32))
    assert pred.shape == (N,) and target.shape == (N,)

    order = np.argsort(-target, kind="stable")  # matches jnp stable argsort
    sp = pred[order]
    u = sp[H - 1:: -1]  # sp[H-1-t]
    v = sp[H:]          # sp[H+t]

    strict = np.triu(np.ones((P, P), np.float32), 1)  # [q,p]=1 iff q<p
    from ml_dtypes import bfloat16 as _bf
    ident = np.eye(P, dtype=np.float32).astype(_bf)  # 0/1: exact in bf16
    ones_col = np.ones((P, 1), np.float32)

    in_maps = []
    for k in range(NCORES):
        hc = np.zeros((P, NCORES), np.float32)
        hc[:, k] = 1.0
        mask = np.zeros((NCORES, P), np.float32)
        mask[:k, :] = 1.0
        in_maps.append({
            "u_in": np.ascontiguousarray(u[k * B:(k + 1) * B].reshape(P, C)),
            "v_in": np.ascontiguousarray(v[k * B:(k + 1) * B].reshape(P, C)),
            "hotcol": hc,
            "maskbc": mask,
            "strict": strict,
            "ident": ident,
            "ones_col": ones_col,
            "neg_lbase": np.full((P, 1), -2.0 * k * B, np.float32),
        })
    return in_maps, u, v


def _run(in_maps, trace=False):
    from concourse import bass_utils
    return bass_utils.run_bass_kernel_spmd(
        _get_nc(), in_maps, list(range(NCORES)), trace=trace
    )


def _finish(res, u, v):
    partials = [r["partial"].reshape(()) for r in res.results]
    lnsum = np.sum(np.asarray(partials, dtype=np.float64))
    log_num = np.sum(u.astype(np.float64)) - np.sum(v.astype(np.float64))
    loss = np.float32(lnsum - log_num)
    return np.asarray(loss, dtype=np.float32).reshape(())


def kernel(pred, target):
    in_maps, u, v = _make_in_maps(pred, target)
    res = _run(in_maps)
    return _finish(res, u, v)


def kernel_traced(pred, target):
    in_maps, u, v = _make_in_maps(pred, target)
    res = _run(in_maps, trace=True)
    return _finish(res, u, v), res


# revision 13
# speedup vs baseline: 102.5746x; 1.6294x over previous
# ListFold loss (exponential transform, beta=1) on 8 Trainium2 NeuronCores.
#
# Math: with sp = pred sorted by target descending, the reference computes
#   loss = sum_i log(den_i) - (sp[i] - sp[n-1-i]),  i in [0, n/2)
#   den_i = (cp[n-i]-cp[i]) * (cm[n-i]-cm[i]) - (n-2i)
# where cp/cm are prefix sums of exp(+-sp). Re-indexing from the middle
# outward with t = n/2-1-i, u[t] = sp[n/2-1-t], v[t] = sp[n/2+t]:
#   s_plus(t)  = cumsum_incl(exp(u)+exp(v))[t]      (= cp[n-i]-cp[i])
#   s_minus(t) = cumsum_incl(exp(-u)+exp(-v))[t]
#   loss = sum_t log(s_plus*s_minus - (2t+2)) - (u[t]-v[t])
# Window sums are exact (no differencing of large prefix sums) and, by
# Cauchy-Schwarz, s_plus*s_minus >= L^2, so den >= L(L-1) > 0: the bf16
# element streams below cannot produce a negative log argument. The
# log_num part enters only through a global sum: sum_t (u-v) is computed
# on the host during unshard (two block sums, no device work).
#
# Sharding: the pair index t is split into 8 contiguous blocks, one per
# core, laid out [128 partitions x 4096] partition-major. Each core scans
# its block chunk-by-chunk along the free axis (tensor_tensor_scan chained
# through the previous chunk's last column), resolves the partition-axis
# carry with a strict-triangular matmul, and the cross-core carry with an
# [8,10] AllGather of per-(core,chunk) totals (scan-style carry
# exchange). Chunk totals ride accum_out on the wp/wm pair-sum STTs; the
# partition/chunk folding happens inside the PE matmuls (hot path:
# ones^T @ aw for the trigger, strict @ aw + mask @ allgather for the
# carry), so the collective trigger only waits on the last pair-sum - on
# the slowest-starting core every cycle of trigger delay lands directly
# on the collective's completion time. Edge chunks are half-width so the
# first exp starts sooner and the last pair-sum ends sooner. Elementwise
# streams are bf16 (2x DVE tensor_tensor; scan state is fp32 internally
# per the ISA). Phase B applies the carry on the otherwise-idle PE as two
# diag(carry) matmuls accumulated in PSUM, leaving one DVE add + one Ln
# per chunk after the collective. Per-core partial losses are summed on
# the host (the unshard step). The argsort is int bookkeeping done on the
# host while sharding (XLA cannot sort on trn2 at all).
#
# (A direct peer remote_dma_broadcast exchange instead of the collective
# was probe-verified functional but takes 6-8 ms under this runtime's
# host-emulated SWDGE path, so the hardware collective stays.)

import numpy as np

N = 8388608
H = N // 2          # pairs
NCORES = 8
B = H // NCORES     # pairs per core
P = 128
C = B // P          # 4096 free-dim columns
FLIST = [512, 1024, 1024, 1024, 512]   # phase chunk widths
NCH = len(FLIST)
OFFS = [sum(FLIST[:i]) for i in range(NCH)]
assert sum(FLIST) == C
KW = 2 * NCH        # aw16 width: wp cols then wm cols

_CACHE = {}


def _build_nc():
    import concourse.bacc as bacc
    import concourse.mybir as mybir
    import concourse.tile as tile

    dt = mybir.dt
    f32 = dt.float32
    bf16 = dt.bfloat16
    Alu = mybir.AluOpType
    Act = mybir.ActivationFunctionType

    nc = bacc.Bacc("TRN2", target_bir_lowering=False, debug=False,
                   num_devices=NCORES)

    u_in = nc.dram_tensor("u_in", [P, C], f32, kind="ExternalInput").ap()
    v_in = nc.dram_tensor("v_in", [P, C], f32, kind="ExternalInput").ap()
    maskbc = nc.dram_tensor("maskbc", [NCORES, P], f32, kind="ExternalInput").ap()
    strict = nc.dram_tensor("strict", [P, P], f32, kind="ExternalInput").ap()
    ident = nc.dram_tensor("ident", [P, P], bf16, kind="ExternalInput").ap()
    ones_col = nc.dram_tensor("ones_col", [P, 1], f32, kind="ExternalInput").ap()
    neg_lbase = nc.dram_tensor("neg_lbase", [P, 1], f32, kind="ExternalInput").ap()
    out_part = nc.dram_tensor("partial", [1, 1], f32, kind="ExternalOutput").ap()

    with tile.TileContext(nc) as tc:
        with (
            tc.tile_pool(name="const", bufs=1) as constp,
            tc.tile_pool(name="big", bufs=1) as bigp,
            tc.tile_pool(name="work", bufs=2) as workp,
            tc.tile_pool(name="small", bufs=1) as smallp,
            tc.tile_pool(name="acc", bufs=1) as accp,
            tc.tile_pool(name="psum", bufs=1, space="PSUM") as psump,
            tc.tile_pool(name="psumB", bufs=2, space="PSUM") as psumbp,
            tc.tile_pool(name="dram", bufs=1, space="DRAM") as dramp,
        ):
            strict_t = constp.tile([P, P], f32, tag="strict")
            ident_t = constp.tile([P, P], bf16, tag="ident")
            maskbc_t = constp.tile([NCORES, P], f32, tag="maskbc")
            ones_col_t = constp.tile([P, 1], f32, tag="ones_col")
            neg_lbase_t = constp.tile([P, 1], f32, tag="neg_lbase")

            # L(t_local) = 2*(p*C + c) + 2; bf16 rounding of L is harmless:
            # den >= L(L-1) makes the relative den error <= 2^-9 * L/(L-1).
            iota_t = bigp.tile([P, C], bf16, tag="iota")
            nc.gpsimd.iota(iota_t[:], pattern=[[2, C]], base=2,
                           channel_multiplier=2 * C,
                           allow_small_or_imprecise_dtypes=True)

            wp_t = bigp.tile([P, C], bf16, tag="wp")   # exp(u)+exp(v)
            wm_t = bigp.tile([P, C], bf16, tag="wm")   # exp(-u)+exp(-v)
            sp_t = bigp.tile([P, C], bf16, tag="sp")   # running scan of wp
            sm_t = bigp.tile([P, C], bf16, tag="sm")   # running scan of wm
            x1_t = bigp.tile([P, C], bf16, tag="x1")   # sp*sm - iota

            aw = accp.tile([P, KW], f32, tag="aw")     # chunk row totals
            aln = accp.tile([P, NCH], f32, tag="aln")  # chunk row sums of ln

            # ---- phase A: exps, pair sums (with accum -> chunk totals),
            # chained chunk scans, carry-independent x1 ----
            for c in range(NCH):
                F = FLIST[c]
                o = OFFS[c]
                cs = slice(o, o + F)
                u_t = workp.tile([P, F], f32, tag=f"u{F}")
                v_t = workp.tile([P, F], f32, tag=f"v{F}")
                nc.sync.dma_start(u_t[:], u_in[:, cs])
                nc.sync.dma_start(v_t[:], v_in[:, cs])

                eu = workp.tile([P, F], bf16, tag=f"eu{F}")
                ev = workp.tile([P, F], bf16, tag=f"ev{F}")
                emu = workp.tile([P, F], bf16, tag=f"emu{F}")
                emv = workp.tile([P, F], bf16, tag=f"emv{F}")
                nc.scalar.activation(eu[:], u_t[:], Act.Exp)
                nc.scalar.activation(ev[:], v_t[:], Act.Exp)
                nc.scalar.activation(emu[:], u_t[:], Act.Exp, scale=-1.0)
                nc.scalar.activation(emv[:], v_t[:], Act.Exp, scale=-1.0)

                # pair sums with chunk-total accumulators riding along
                nc.vector.scalar_tensor_tensor(
                    out=wp_t[:, cs], in0=eu[:], scalar=0.0, in1=ev[:],
                    op0=Alu.add, op1=Alu.add, accum_out=aw[:, c:c + 1])
                nc.vector.scalar_tensor_tensor(
                    out=wm_t[:, cs], in0=emu[:], scalar=0.0, in1=emv[:],
                    op0=Alu.add, op1=Alu.add,
                    accum_out=aw[:, NCH + c:NCH + c + 1])

                ip = 0.0 if c == 0 else sp_t[:, o - 1:o]
                im = 0.0 if c == 0 else sm_t[:, o - 1:o]
                nc.vector.tensor_tensor_scan(
                    sp_t[:, cs], wp_t[:, cs], wp_t[:, cs], ip,
                    Alu.add, Alu.bypass)
                nc.vector.tensor_tensor_scan(
                    sm_t[:, cs], wm_t[:, cs], wm_t[:, cs], im,
                    Alu.add, Alu.bypass)

                prod = workp.tile([P, F], bf16, tag=f"prod{F}")
                nc.vector.tensor_mul(prod[:], sp_t[:, cs], sm_t[:, cs])
                nc.vector.tensor_sub(x1_t[:, cs], prod[:], iota_t[:, cs])

            # consts are needed from the carry stage on - issue their DMAs
            # after the phase-A loads so chunk 0 starts sooner
            nc.sync.dma_start(strict_t[:], strict)
            nc.sync.dma_start(ident_t[:], ident)
            nc.sync.dma_start(maskbc_t[:], maskbc)
            nc.sync.dma_start(ones_col_t[:], ones_col)
            nc.sync.dma_start(neg_lbase_t[:], neg_lbase)

            # ---- trigger path: my totals row = ones^T @ aw, then gather ----
            contrib_ps = psump.tile([1, KW], f32, tag="contrib")
            nc.tensor.matmul(contrib_ps[:], ones_col_t[:], aw[:],
                             start=True, stop=True)
            contrib_sb = smallp.tile([1, KW], f32, tag="contrib_sb")
            nc.scalar.copy(contrib_sb[:], contrib_ps[:])

            cc_in = dramp.tile([1, KW], f32, tag="cc_in")
            cc_out = dramp.tile([NCORES, KW], f32, tag="cc_out")
            nc.sync.dma_start(cc_in[:], contrib_sb[:])
            nc.gpsimd.collective_compute(
                "AllGather", Alu.bypass,
                replica_groups=[list(range(NCORES))],
                ins=[cc_in.opt()], outs=[cc_out.opt()])
            allt = smallp.tile([NCORES, KW], f32, tag="allt")
            nc.sync.dma_start(allt[:], cc_out[:])

            # carry = strict-local partition prefix + earlier cores' totals,
            # both PSUM-accumulated, then folded over chunks by two reduces
            carry_ps = psump.tile([P, KW], f32, tag="carry")
            nc.tensor.matmul(carry_ps[:], strict_t[:], aw[:],
                             start=True, stop=False)
            nc.tensor.matmul(carry_ps[:], maskbc_t[:], allt[:],
                             start=False, stop=True)
            carry_sb = smallp.tile([P, 2], f32, tag="carry_sb")
            nc.vector.tensor_reduce(carry_sb[:, 0:1], carry_ps[:, 0:NCH],
                                    axis=mybir.AxisListType.X, op=Alu.add)
            nc.vector.tensor_reduce(carry_sb[:, 1:2], carry_ps[:, NCH:KW],
                                    axis=mybir.AxisListType.X, op=Alu.add)

            # warm the Ln table while ACT is idle (input: any ready f32)
            lnwarm = smallp.tile([P, 1], f32, tag="lnwarm")
            nc.scalar.activation(lnwarm[:], aw[:, 0:1], Act.Ln)

            # diag(Cp), diag(Cm) as bf16 lhsT for the phase-B PE matmuls
            dcp = smallp.tile([P, P], bf16, tag="dcp")
            dcm = smallp.tile([P, P], bf16, tag="dcm")
            nc.vector.tensor_scalar(
                out=dcp[:], in0=ident_t[:], scalar1=carry_sb[:, 0:1],
                scalar2=None, op0=Alu.mult)
            nc.vector.tensor_scalar(
                out=dcm[:], in0=ident_t[:], scalar1=carry_sb[:, 1:2],
                scalar2=None, op0=Alu.mult)

            # bias = Cp*Cm - 2kB (per-partition scalar for the Ln)
            cpcm = smallp.tile([P, 1], f32, tag="cpcm")
            nc.vector.tensor_scalar(
                out=cpcm[:], in0=carry_sb[:, 0:1], scalar1=carry_sb[:, 1:2],
                scalar2=None, op0=Alu.mult)
            bias_t = smallp.tile([P, 1], f32, tag="bias_t")
            nc.vector.tensor_add(bias_t[:], cpcm[:], neg_lbase_t[:])

            # ---- phase B: den = x1 + Cp*sm + Cm*sp + (CpCm - 2kB), log.
            # Carry products on the PE (PSUM-accumulated, one bank per
            # 512-col sub-tile); DVE adds x1; ACT takes the log. ----
            HB = 512
            for c in range(NCH):
                F = FLIST[c]
                o = OFFS[c]
                t2 = workp.tile([P, F], bf16, tag=f"t2{F}")
                for h in range(F // HB):
                    hs = slice(o + h * HB, o + (h + 1) * HB)
                    ps_h = psumbp.tile([P, HB], f32, tag="psB")
                    nc.tensor.matmul(ps_h[:], dcp[:], sm_t[:, hs],
                                     start=True, stop=False)
                    nc.tensor.matmul(ps_h[:], dcm[:], sp_t[:, hs],
                                     start=False, stop=True)
                    nc.vector.tensor_add(t2[:, h * HB:(h + 1) * HB],
                                         x1_t[:, hs], ps_h[:])
                ln_o = workp.tile([P, F], bf16, tag=f"ln{F}")
                nc.scalar.activation(ln_o[:], t2[:], Act.Ln,
                                     bias=bias_t[:],
                                     accum_out=aln[:, c:c + 1])

            rll = smallp.tile([P, 1], f32, tag="rll")
            nc.vector.tensor_reduce(rll[:], aln[:], axis=mybir.AxisListType.X,
                                    op=Alu.add)

            part_ps = psump.tile([1, 1], f32, tag="part")
            nc.tensor.matmul(part_ps[:], ones_col_t[:], rll[:],
                             start=True, stop=True)
            part_sb = smallp.tile([1, 1], f32, tag="part_sb")
            nc.scalar.copy(part_sb[:], part_ps[:])
            nc.sync.dma_start(out_part, part_sb[:])

    nc.compile()
    return nc


def _get_nc():
    if "nc" not in _CACHE:
        _CACHE["nc"] = _build_nc()
    return _CACHE["nc"]


def _make_in_maps(pred, target):
    pred = np.ascontiguousarray(np.asarray(pred, dtype=np.float32))
    target = np.ascontiguousarray(np.asarray(target, dtype=np.float32))
    assert pred.shape == (N,) and target.shape == (N,)

    order = np.argsort(-target, kind="stable")  # matches jnp stable argsort
    sp = pred[order]
    u = sp[H - 1:: -1]  # sp[H-1-t]
    v = sp[H:]          # sp[H+t]

    strict = np.triu(np.ones((P, P), np.float32), 1)  # [q,p]=1 iff q<p
    from ml_dtypes import bfloat16 as _bf
    ident = np.eye(P, dtype=np.float32).astype(_bf)  # 0/1: exact in bf16
    ones_col = np.ones((P, 1), np.float32)

    in_maps = []
    for k in range(NCORES):
        mask = np.zeros((NCORES, P), np.float32)
        mask[:k, :] = 1.0
        in_maps.append({
            "u_in": np.ascontiguousarray(u[k * B:(k + 1) * B].reshape(P, C)),
            "v_in": np.ascontiguousarray(v[k * B:(k + 1) * B].reshape(P, C)),
            "maskbc": mask,
            "strict": strict,
            "ident": ident,
            "ones_col": ones_col,
            "neg_lbase": np.full((P, 1), -2.0 * k * B, np.float32),
        })
    return in_maps, u, v


def _run(in_maps, trace=False):
    from concourse import bass_utils
    return bass_utils.run_bass_kernel_spmd(
        _get_nc(), in_maps, list(range(NCORES)), trace=trace
    )


def _finish(res, u, v):
    partials = [r["partial"].reshape(()) for r in res.results]
    lnsum = np.sum(np.asarray(partials, dtype=np.float64))
    log_num = np.sum(u.astype(np.float64)) - np.sum(v.astype(np.float64))
    loss = np.float32(lnsum - log_num)
    return np.asarray(loss, dtype=np.float32).reshape(())


def kernel(pred, target):
    in_maps, u, v = _make_in_maps(pred, target)
    res = _run(in_maps)
    return _finish(res, u, v)


def kernel_traced(pred, target):
    in_maps, u, v = _make_in_maps(pred, target)
    res = _run(in_maps, trace=True)
    return _finish(res, u, v), res


# revision 15
# speedup vs baseline: 107.2160x; 1.0452x over previous
# ListFold loss (exponential transform, beta=1) on 8 Trainium2 NeuronCores.
#
# Math: with sp = pred sorted by target descending, the reference computes
#   loss = sum_i log(den_i) - (sp[i] - sp[n-1-i]),  i in [0, n/2)
#   den_i = (cp[n-i]-cp[i]) * (cm[n-i]-cm[i]) - (n-2i)
# where cp/cm are prefix sums of exp(+-sp). Re-indexing from the middle
# outward with t = n/2-1-i, u[t] = sp[n/2-1-t], v[t] = sp[n/2+t]:
#   s_plus(t)  = cumsum_incl(exp(u)+exp(v))[t]      (= cp[n-i]-cp[i])
#   s_minus(t) = cumsum_incl(exp(-u)+exp(-v))[t]
#   loss = sum_t log(s_plus*s_minus - (2t+2)) - (u[t]-v[t])
# Window sums are exact (no differencing of large prefix sums) and, by
# Cauchy-Schwarz, s_plus*s_minus >= L^2, so den >= L(L-1) > 0: the bf16
# element streams below cannot produce a negative log argument. The
# log_num part enters only through a global sum: sum_t (u-v) is computed
# on the host during unshard (two block sums, no device work).
#
# Sharding: the pair index t is split into 8 contiguous blocks, one per
# core, laid out [128 partitions x 4096] partition-major. Each core scans
# its block chunk-by-chunk along the free axis (tensor_tensor_scan chained
# through the previous chunk's last column), resolves the partition-axis
# carry with a strict-triangular matmul, and the cross-core carry with an
# [8,10] AllGather of per-(core,chunk) totals (scan-style carry
# exchange). Chunk totals ride accum_out on the wp/wm pair-sum STTs; the
# partition/chunk folding happens inside the PE matmuls (hot path:
# ones^T @ aw for the trigger, strict @ aw + mask @ allgather for the
# carry), so the collective trigger only waits on the last pair-sum - on
# the slowest-starting core every cycle of trigger delay lands directly
# on the collective's completion time. Edge chunks are half-width so the
# first exp starts sooner and the last pair-sum ends sooner. Elementwise
# streams are bf16 (2x DVE tensor_tensor; scan state is fp32 internally
# per the ISA). Phase B applies the carry on the otherwise-idle PE as two
# diag(carry) matmuls accumulated in PSUM, leaving one DVE add + one Ln
# per chunk after the collective. Per-core partial losses are summed on
# the host (the unshard step). The argsort is int bookkeeping done on the
# host while sharding (XLA cannot sort on trn2 at all).
#
# (A direct peer remote_dma_broadcast exchange instead of the collective
# was probe-verified functional but takes 6-8 ms under this runtime's
# host-emulated SWDGE path, so the hardware collective stays.)

import numpy as np

N = 8388608
H = N // 2          # pairs
NCORES = 8
B = H // NCORES     # pairs per core
P = 128
C = B // P          # 4096 free-dim columns
FLIST = [512, 1024, 1024, 1024, 512]   # phase chunk widths
NCH = len(FLIST)
OFFS = [sum(FLIST[:i]) for i in range(NCH)]
assert sum(FLIST) == C
KW = 2 * NCH        # aw16 width: wp cols then wm cols

_CACHE = {}


def _build_nc():
    import concourse.bacc as bacc
    import concourse.mybir as mybir
    import concourse.tile as tile

    dt = mybir.dt
    f32 = dt.float32
    bf16 = dt.bfloat16
    Alu = mybir.AluOpType
    Act = mybir.ActivationFunctionType

    nc = bacc.Bacc("TRN2", target_bir_lowering=False, debug=False,
                   num_devices=NCORES)

    u_in = nc.dram_tensor("u_in", [P, C], f32, kind="ExternalInput").ap()
    v_in = nc.dram_tensor("v_in", [P, C], f32, kind="ExternalInput").ap()
    maskbc = nc.dram_tensor("maskbc", [NCORES, P], f32, kind="ExternalInput").ap()
    strict = nc.dram_tensor("strict", [P, P], f32, kind="ExternalInput").ap()
    ident = nc.dram_tensor("ident", [P, P], bf16, kind="ExternalInput").ap()
    ones_col = nc.dram_tensor("ones_col", [P, 1], f32, kind="ExternalInput").ap()
    neg_lbase = nc.dram_tensor("neg_lbase", [P, 1], f32, kind="ExternalInput").ap()
    out_part = nc.dram_tensor("partial", [1, 1], f32, kind="ExternalOutput").ap()

    with tile.TileContext(nc) as tc:
        with (
            tc.tile_pool(name="const", bufs=1) as constp,
            tc.tile_pool(name="big", bufs=1) as bigp,
            tc.tile_pool(name="work", bufs=2) as workp,
            tc.tile_pool(name="small", bufs=1) as smallp,
            tc.tile_pool(name="acc", bufs=1) as accp,
            tc.tile_pool(name="psum", bufs=1, space="PSUM") as psump,
            tc.tile_pool(name="psumB", bufs=2, space="PSUM") as psumbp,
            tc.tile_pool(name="dram", bufs=1, space="DRAM") as dramp,
        ):
            strict_t = constp.tile([P, P], f32, tag="strict")
            ident_t = constp.tile([P, P], bf16, tag="ident")
            maskbc_t = constp.tile([NCORES, P], f32, tag="maskbc")
            ones_col_t = constp.tile([P, 1], f32, tag="ones_col")
            neg_lbase_t = constp.tile([P, 1], f32, tag="neg_lbase")

            # L(t_local) = 2*(p*C + c) + 2; bf16 rounding of L is harmless:
            # den >= L(L-1) makes the relative den error <= 2^-9 * L/(L-1).
            iota_t = bigp.tile([P, C], bf16, tag="iota")
            nc.gpsimd.iota(iota_t[:], pattern=[[2, C]], base=2,
                           channel_multiplier=2 * C,
                           allow_small_or_imprecise_dtypes=True)

            wp_t = bigp.tile([P, C], bf16, tag="wp")   # exp(u)+exp(v)
            wm_t = bigp.tile([P, C], bf16, tag="wm")   # exp(-u)+exp(-v)
            sp_t = bigp.tile([P, C], bf16, tag="sp")   # running scan of wp
            sm_t = bigp.tile([P, C], bf16, tag="sm")   # running scan of wm
            x1_t = bigp.tile([P, C], bf16, tag="x1")   # sp*sm - iota

            aw = accp.tile([P, KW], f32, tag="aw")     # chunk row totals
            aln = accp.tile([P, NCH], f32, tag="aln")  # chunk row sums of ln

            # ---- phase A pass 1: exps + pair sums (with accum -> chunk
            # totals). Emitted before any scan so the trigger-critical
            # STTs sit at the front of the DVE queue; the scans have a
            # whole collective window of slack. ----
            for c in range(NCH):
                F = FLIST[c]
                o = OFFS[c]
                cs = slice(o, o + F)
                u_t = workp.tile([P, F], f32, tag=f"u{F}")
                v_t = workp.tile([P, F], f32, tag=f"v{F}")
                nc.sync.dma_start(u_t[:], u_in[:, cs])
                nc.sync.dma_start(v_t[:], v_in[:, cs])

                eu = workp.tile([P, F], bf16, tag=f"eu{F}")
                ev = workp.tile([P, F], bf16, tag=f"ev{F}")
                emu = workp.tile([P, F], bf16, tag=f"emu{F}")
                emv = workp.tile([P, F], bf16, tag=f"emv{F}")
                nc.scalar.activation(eu[:], u_t[:], Act.Exp)
                nc.scalar.activation(ev[:], v_t[:], Act.Exp)
                nc.scalar.activation(emu[:], u_t[:], Act.Exp, scale=-1.0)
                nc.scalar.activation(emv[:], v_t[:], Act.Exp, scale=-1.0)

                nc.vector.scalar_tensor_tensor(
                    out=wp_t[:, cs], in0=eu[:], scalar=0.0, in1=ev[:],
                    op0=Alu.add, op1=Alu.add, accum_out=aw[:, c:c + 1])
                nc.vector.scalar_tensor_tensor(
                    out=wm_t[:, cs], in0=emu[:], scalar=0.0, in1=emv[:],
                    op0=Alu.add, op1=Alu.add,
                    accum_out=aw[:, NCH + c:NCH + c + 1])

            # ---- phase A pass 2: chained chunk scans + carry-independent
            # x1 (fills the collective window) ----
            for c in range(NCH):
                F = FLIST[c]
                o = OFFS[c]
                cs = slice(o, o + F)
                ip = 0.0 if c == 0 else sp_t[:, o - 1:o]
                im = 0.0 if c == 0 else sm_t[:, o - 1:o]
                nc.vector.tensor_tensor_scan(
                    sp_t[:, cs], wp_t[:, cs], wp_t[:, cs], ip,
                    Alu.add, Alu.bypass)
                nc.vector.tensor_tensor_scan(
                    sm_t[:, cs], wm_t[:, cs], wm_t[:, cs], im,
                    Alu.add, Alu.bypass)

                prod = workp.tile([P, F], bf16, tag=f"prod{F}")
                nc.vector.tensor_mul(prod[:], sp_t[:, cs], sm_t[:, cs])
                nc.vector.tensor_sub(x1_t[:, cs], prod[:], iota_t[:, cs])

            # consts are needed from the carry stage on - issue their DMAs
            # after the phase-A loads so chunk 0 starts sooner
            nc.sync.dma_start(strict_t[:], strict)
            nc.sync.dma_start(ident_t[:], ident)
            nc.sync.dma_start(maskbc_t[:], maskbc)
            nc.sync.dma_start(ones_col_t[:], ones_col)
            nc.sync.dma_start(neg_lbase_t[:], neg_lbase)

            # ---- trigger path: my totals row = ones^T @ aw, then gather ----
            contrib_ps = psump.tile([1, KW], f32, tag="contrib")
            nc.tensor.matmul(contrib_ps[:], ones_col_t[:], aw[:],
                             start=True, stop=True)
            contrib_sb = smallp.tile([1, KW], f32, tag="contrib_sb")
            nc.scalar.copy(contrib_sb[:], contrib_ps[:])

            cc_in = dramp.tile([1, KW], f32, tag="cc_in")
            cc_out = dramp.tile([NCORES, KW], f32, tag="cc_out")
            nc.sync.dma_start(cc_in[:], contrib_sb[:])
            nc.gpsimd.collective_compute(
                "AllGather", Alu.bypass,
                replica_groups=[list(range(NCORES))],
                ins=[cc_in.opt()], outs=[cc_out.opt()])
            allt = smallp.tile([NCORES, KW], f32, tag="allt")
            nc.sync.dma_start(allt[:], cc_out[:])

            # carry = strict-local partition prefix + earlier cores' totals,
            # both PSUM-accumulated, then folded over chunks by two reduces
            carry_ps = psump.tile([P, KW], f32, tag="carry")
            nc.tensor.matmul(carry_ps[:], strict_t[:], aw[:],
                             start=True, stop=False)
            nc.tensor.matmul(carry_ps[:], maskbc_t[:], allt[:],
                             start=False, stop=True)
            carry_sb = smallp.tile([P, 2], f32, tag="carry_sb")
            nc.vector.tensor_reduce(carry_sb[:, 0:1], carry_ps[:, 0:NCH],
                                    axis=mybir.AxisListType.X, op=Alu.add)
            nc.vector.tensor_reduce(carry_sb[:, 1:2], carry_ps[:, NCH:KW],
                                    axis=mybir.AxisListType.X, op=Alu.add)

            # warm the Ln table while ACT is idle (input: any ready f32)
            lnwarm = smallp.tile([P, 1], f32, tag="lnwarm")
            nc.scalar.activation(lnwarm[:], aw[:, 0:1], Act.Ln)

            # diag(Cp), diag(Cm) as bf16 lhsT for the phase-B PE matmuls
            dcp = smallp.tile([P, P], bf16, tag="dcp")
            dcm = smallp.tile([P, P], bf16, tag="dcm")
            nc.vector.tensor_scalar(
                out=dcp[:], in0=ident_t[:], scalar1=carry_sb[:, 0:1],
                scalar2=None, op0=Alu.mult)
            nc.vector.tensor_scalar(
                out=dcm[:], in0=ident_t[:], scalar1=carry_sb[:, 1:2],
                scalar2=None, op0=Alu.mult)

            # bias = Cp*Cm - 2kB (per-partition scalar for the Ln)
            cpcm = smallp.tile([P, 1], f32, tag="cpcm")
            nc.vector.tensor_scalar(
                out=cpcm[:], in0=carry_sb[:, 0:1], scalar1=carry_sb[:, 1:2],
                scalar2=None, op0=Alu.mult)
            bias_t = smallp.tile([P, 1], f32, tag="bias_t")
            nc.vector.tensor_add(bias_t[:], cpcm[:], neg_lbase_t[:])

            # ---- phase B: den = x1 + Cp*sm + Cm*sp + (CpCm - 2kB), log.
            # Carry products on the PE (PSUM-accumulated, one bank per
            # 512-col sub-tile); DVE adds x1; ACT takes the log. ----
            HB = 512
            for c in range(NCH):
                F = FLIST[c]
                o = OFFS[c]
                t2 = workp.tile([P, F], bf16, tag=f"t2{F}")
                for h in range(F // HB):
                    hs = slice(o + h * HB, o + (h + 1) * HB)
                    ps_h = psumbp.tile([P, HB], f32, tag="psB")
                    nc.tensor.matmul(ps_h[:], dcp[:], sm_t[:, hs],
                                     start=True, stop=False)
                    nc.tensor.matmul(ps_h[:], dcm[:], sp_t[:, hs],
                                     start=False, stop=True)
                    nc.vector.tensor_add(t2[:, h * HB:(h + 1) * HB],
                                         x1_t[:, hs], ps_h[:])
                ln_o = workp.tile([P, F], bf16, tag=f"ln{F}")
                nc.scalar.activation(ln_o[:], t2[:], Act.Ln,
                                     bias=bias_t[:],
                                     accum_out=aln[:, c:c + 1])

            rll = smallp.tile([P, 1], f32, tag="rll")
            nc.vector.tensor_reduce(rll[:], aln[:], axis=mybir.AxisListType.X,
                                    op=Alu.add)

            part_ps = psump.tile([1, 1], f32, tag="part")
            nc.tensor.matmul(part_ps[:], ones_col_t[:], rll[:],
                             start=True, stop=True)
            part_sb = smallp.tile([1, 1], f32, tag="part_sb")
            nc.scalar.copy(part_sb[:], part_ps[:])
            nc.sync.dma_start(out_part, part_sb[:])

    nc.compile()
    return nc


def _get_nc():
    if "nc" not in _CACHE:
        _CACHE["nc"] = _build_nc()
    return _CACHE["nc"]


def _make_in_maps(pred, target):
    pred = np.ascontiguousarray(np.asarray(pred, dtype=np.float32))
    target = np.ascontiguousarray(np.asarray(target, dtype=np.float32))
    assert pred.shape == (N,) and target.shape == (N,)

    order = np.argsort(-target, kind="stable")  # matches jnp stable argsort
    sp = pred[order]
    u = sp[H - 1:: -1]  # sp[H-1-t]
    v = sp[H:]          # sp[H+t]

    strict = np.triu(np.ones((P, P), np.float32), 1)  # [q,p]=1 iff q<p
    from ml_dtypes import bfloat16 as _bf
    ident = np.eye(P, dtype=np.float32).astype(_bf)  # 0/1: exact in bf16
    ones_col = np.ones((P, 1), np.float32)

    in_maps = []
    for k in range(NCORES):
        mask = np.zeros((NCORES, P), np.float32)
        mask[:k, :] = 1.0
        in_maps.append({
            "u_in": np.ascontiguousarray(u[k * B:(k + 1) * B].reshape(P, C)),
            "v_in": np.ascontiguousarray(v[k * B:(k + 1) * B].reshape(P, C)),
            "maskbc": mask,
            "strict": strict,
            "ident": ident,
            "ones_col": ones_col,
            "neg_lbase": np.full((P, 1), -2.0 * k * B, np.float32),
        })
    return in_maps, u, v


def _run(in_maps, trace=False):
    from concourse import bass_utils
    return bass_utils.run_bass_kernel_spmd(
        _get_nc(), in_maps, list(range(NCORES)), trace=trace
    )


def _finish(res, u, v):
    partials = [r["partial"].reshape(()) for r in res.results]
    lnsum = np.sum(np.asarray(partials, dtype=np.float64))
    log_num = np.sum(u.astype(np.float64)) - np.sum(v.astype(np.float64))
    loss = np.float32(lnsum - log_num)
    return np.asarray(loss, dtype=np.float32).reshape(())


def kernel(pred, target):
    in_maps, u, v = _make_in_maps(pred, target)
    res = _run(in_maps)
    return _finish(res, u, v)


def kernel_traced(pred, target):
    in_maps, u, v = _make_in_maps(pred, target)
    res = _run(in_maps, trace=True)
    return _finish(res, u, v), res


# revision 16
# speedup vs baseline: 122.6244x; 1.1437x over previous
# ListFold loss (exponential transform, beta=1) on 8 Trainium2 NeuronCores.
#
# Math: with sp = pred sorted by target descending, the reference computes
#   loss = sum_i log(den_i) - (sp[i] - sp[n-1-i]),  i in [0, n/2)
#   den_i = (cp[n-i]-cp[i]) * (cm[n-i]-cm[i]) - (n-2i)
# where cp/cm are prefix sums of exp(+-sp). Re-indexing from the middle
# outward with t = n/2-1-i, u[t] = sp[n/2-1-t], v[t] = sp[n/2+t]:
#   s_plus(t)  = cumsum_incl(exp(u)+exp(v))[t]      (= cp[n-i]-cp[i])
#   s_minus(t) = cumsum_incl(exp(-u)+exp(-v))[t]
#   loss = sum_t log(s_plus*s_minus - (2t+2)) - (u[t]-v[t])
# Window sums are exact (no differencing of large prefix sums) and, by
# Cauchy-Schwarz, s_plus*s_minus >= L^2, so den >= L(L-1) > 0: the bf16
# element streams below cannot produce a negative log argument. The
# log_num part enters only through a global sum: sum_t (u-v) is computed
# on the host during unshard (two block sums, no device work).
#
# Sharding: the pair index t is split into 8 contiguous blocks, one per
# core, laid out [128 partitions x 4096] partition-major. Each core scans
# its block chunk-by-chunk along the free axis (tensor_tensor_scan chained
# through the previous chunk's last column), resolves the partition-axis
# carry with a strict-triangular matmul, and the cross-core carry with an
# [8,10] AllGather of per-(core,chunk) totals (scan-style carry
# exchange). Chunk totals ride accum_out on the wp/wm pair-sum STTs; the
# partition/chunk folding happens inside the PE matmuls (hot path:
# ones^T @ aw for the trigger, strict @ aw + mask @ allgather for the
# carry), so the collective trigger only waits on the last pair-sum - on
# the slowest-starting core every cycle of trigger delay lands directly
# on the collective's completion time. Edge chunks are half-width so the
# first exp starts sooner and the last pair-sum ends sooner. Elementwise
# streams are bf16 (2x DVE tensor_tensor; scan state is fp32 internally
# per the ISA). Phase B applies the carry on the otherwise-idle PE as two
# diag(carry) matmuls accumulated in PSUM, leaving one DVE add + one Ln
# per chunk after the collective. Per-core partial losses are summed on
# the host (the unshard step). The argsort is int bookkeeping done on the
# host while sharding (XLA cannot sort on trn2 at all).
#
# (A direct peer remote_dma_broadcast exchange instead of the collective
# was probe-verified functional but takes 6-8 ms under this runtime's
# host-emulated SWDGE path, so the hardware collective stays.)

import numpy as np

N = 8388608
H = N // 2          # pairs
NCORES = 8
B = H // NCORES     # pairs per core
P = 128
C = B // P          # 4096 free-dim columns
FLIST = [512, 1024, 1024, 1024, 512]   # phase chunk widths
NCH = len(FLIST)
OFFS = [sum(FLIST[:i]) for i in range(NCH)]
assert sum(FLIST) == C
KW = 2 * NCH        # aw16 width: wp cols then wm cols

_CACHE = {}


def _build_nc():
    import concourse.bacc as bacc
    import concourse.mybir as mybir
    import concourse.tile as tile

    dt = mybir.dt
    f32 = dt.float32
    bf16 = dt.bfloat16
    Alu = mybir.AluOpType
    Act = mybir.ActivationFunctionType

    nc = bacc.Bacc("TRN2", target_bir_lowering=False, debug=False,
                   num_devices=NCORES)

    u_in = nc.dram_tensor("u_in", [P, C], f32, kind="ExternalInput").ap()
    v_in = nc.dram_tensor("v_in", [P, C], f32, kind="ExternalInput").ap()
    maskbc = nc.dram_tensor("maskbc", [NCORES, P], f32, kind="ExternalInput").ap()
    strict = nc.dram_tensor("strict", [P, P], f32, kind="ExternalInput").ap()
    ident = nc.dram_tensor("ident", [P, P], bf16, kind="ExternalInput").ap()
    ones_col = nc.dram_tensor("ones_col", [P, 1], f32, kind="ExternalInput").ap()
    neg_lbase = nc.dram_tensor("neg_lbase", [P, 1], f32, kind="ExternalInput").ap()
    out_part = nc.dram_tensor("partial", [1, 1], f32, kind="ExternalOutput").ap()

    with tile.TileContext(nc) as tc:
        with (
            tc.tile_pool(name="const", bufs=1) as constp,
            tc.tile_pool(name="big", bufs=1) as bigp,
            tc.tile_pool(name="work", bufs=2) as workp,
            tc.tile_pool(name="small", bufs=1) as smallp,
            tc.tile_pool(name="acc", bufs=1) as accp,
            tc.tile_pool(name="psum", bufs=1, space="PSUM") as psump,
            tc.tile_pool(name="psumB", bufs=2, space="PSUM") as psumbp,
            tc.tile_pool(name="dram", bufs=1, space="DRAM") as dramp,
        ):
            strict_t = constp.tile([P, P], f32, tag="strict")
            ident_t = constp.tile([P, P], bf16, tag="ident")
            maskbc_t = constp.tile([NCORES, P], f32, tag="maskbc")
            ones_col_t = constp.tile([P, 1], f32, tag="ones_col")
            neg_lbase_t = constp.tile([P, 1], f32, tag="neg_lbase")

            # L(t_local) = 2*(p*C + c) + 2; bf16 rounding of L is harmless:
            # den >= L(L-1) makes the relative den error <= 2^-9 * L/(L-1).
            iota_t = bigp.tile([P, C], bf16, tag="iota")
            nc.gpsimd.iota(iota_t[:], pattern=[[2, C]], base=2,
                           channel_multiplier=2 * C,
                           allow_small_or_imprecise_dtypes=True)

            wp_t = bigp.tile([P, C], bf16, tag="wp")   # exp(u)+exp(v)
            wm_t = bigp.tile([P, C], bf16, tag="wm")   # exp(-u)+exp(-v)
            sp_t = bigp.tile([P, C], bf16, tag="sp")   # running scan of wp
            sm_t = bigp.tile([P, C], bf16, tag="sm")   # running scan of wm
            x1_t = bigp.tile([P, C], bf16, tag="x1")   # sp*sm - iota

            aw = accp.tile([P, KW], f32, tag="aw")     # chunk row totals
            aln = accp.tile([P, NCH], f32, tag="aln")  # chunk row sums of ln

            # ---- phase A pass 1: exps + pair sums (with accum -> chunk
            # totals). Emitted before any scan so the trigger-critical
            # STTs sit at the front of the DVE queue; the scans have a
            # whole collective window of slack. ----
            for c in range(NCH):
                F = FLIST[c]
                o = OFFS[c]
                cs = slice(o, o + F)
                u_t = workp.tile([P, F], f32, tag=f"u{F}")
                v_t = workp.tile([P, F], f32, tag=f"v{F}")
                nc.sync.dma_start(u_t[:], u_in[:, cs])
                nc.sync.dma_start(v_t[:], v_in[:, cs])

                eu = workp.tile([P, F], bf16, tag=f"eu{F}")
                ev = workp.tile([P, F], bf16, tag=f"ev{F}")
                emu = workp.tile([P, F], bf16, tag=f"emu{F}")
                emv = workp.tile([P, F], bf16, tag=f"emv{F}")
                nc.scalar.activation(eu[:], u_t[:], Act.Exp)
                nc.scalar.activation(ev[:], v_t[:], Act.Exp)
                nc.scalar.activation(emu[:], u_t[:], Act.Exp, scale=-1.0)
                nc.scalar.activation(emv[:], v_t[:], Act.Exp, scale=-1.0)

                nc.vector.scalar_tensor_tensor(
                    out=wp_t[:, cs], in0=eu[:], scalar=0.0, in1=ev[:],
                    op0=Alu.add, op1=Alu.add, accum_out=aw[:, c:c + 1])
                nc.vector.scalar_tensor_tensor(
                    out=wm_t[:, cs], in0=emu[:], scalar=0.0, in1=emv[:],
                    op0=Alu.add, op1=Alu.add,
                    accum_out=aw[:, NCH + c:NCH + c + 1])

            # ---- phase A pass 2: chained chunk scans + carry-independent
            # x1 (fills the collective window) ----
            for c in range(NCH):
                F = FLIST[c]
                o = OFFS[c]
                cs = slice(o, o + F)
                ip = 0.0 if c == 0 else sp_t[:, o - 1:o]
                im = 0.0 if c == 0 else sm_t[:, o - 1:o]
                # data1 is ignored (op1=bypass) but points at the LAST
                # chunk's wm window: a deliberate dependency on the final
                # pair-sum STT so no scan can slip ahead of the
                # trigger-critical STT chain in the DVE queue.
                tail_dep_p = wm_t[:, C - F:C]
                tail_dep_m = wm_t[:, C - F:C]
                nc.vector.tensor_tensor_scan(
                    sp_t[:, cs], wp_t[:, cs], tail_dep_p, ip,
                    Alu.add, Alu.bypass)
                nc.vector.tensor_tensor_scan(
                    sm_t[:, cs], wm_t[:, cs], tail_dep_m, im,
                    Alu.add, Alu.bypass)

                prod = workp.tile([P, F], bf16, tag=f"prod{F}")
                nc.vector.tensor_mul(prod[:], sp_t[:, cs], sm_t[:, cs])
                nc.vector.tensor_sub(x1_t[:, cs], prod[:], iota_t[:, cs])

            # consts are needed from the carry stage on - issue their DMAs
            # after the phase-A loads so chunk 0 starts sooner
            nc.sync.dma_start(strict_t[:], strict)
            nc.sync.dma_start(ident_t[:], ident)
            nc.sync.dma_start(maskbc_t[:], maskbc)
            nc.sync.dma_start(ones_col_t[:], ones_col)
            nc.sync.dma_start(neg_lbase_t[:], neg_lbase)

            # ---- trigger path: my totals row = ones^T @ aw, then gather ----
            contrib_ps = psump.tile([1, KW], f32, tag="contrib")
            nc.tensor.matmul(contrib_ps[:], ones_col_t[:], aw[:],
                             start=True, stop=True)
            contrib_sb = smallp.tile([1, KW], f32, tag="contrib_sb")
            nc.scalar.copy(contrib_sb[:], contrib_ps[:])

            cc_in = dramp.tile([1, KW], f32, tag="cc_in")
            cc_out = dramp.tile([NCORES, KW], f32, tag="cc_out")
            nc.sync.dma_start(cc_in[:], contrib_sb[:])
            nc.gpsimd.collective_compute(
                "AllGather", Alu.bypass,
                replica_groups=[list(range(NCORES))],
                ins=[cc_in.opt()], outs=[cc_out.opt()])
            allt = smallp.tile([NCORES, KW], f32, tag="allt")
            nc.sync.dma_start(allt[:], cc_out[:])

            # carry = strict-local partition prefix + earlier cores' totals,
            # both PSUM-accumulated, then folded over chunks by two reduces
            carry_ps = psump.tile([P, KW], f32, tag="carry")
            nc.tensor.matmul(carry_ps[:], strict_t[:], aw[:],
                             start=True, stop=False)
            nc.tensor.matmul(carry_ps[:], maskbc_t[:], allt[:],
                             start=False, stop=True)
            carry_sb = smallp.tile([P, 2], f32, tag="carry_sb")
            nc.vector.tensor_reduce(carry_sb[:, 0:1], carry_ps[:, 0:NCH],
                                    axis=mybir.AxisListType.X, op=Alu.add)
            nc.vector.tensor_reduce(carry_sb[:, 1:2], carry_ps[:, NCH:KW],
                                    axis=mybir.AxisListType.X, op=Alu.add)

            # warm the Ln table while ACT is idle (input: any ready f32)
            lnwarm = smallp.tile([P, 1], f32, tag="lnwarm")
            nc.scalar.activation(lnwarm[:], aw[:, 0:1], Act.Ln)

            # diag(Cp), diag(Cm) as bf16 lhsT for the phase-B PE matmuls
            dcp = smallp.tile([P, P], bf16, tag="dcp")
            dcm = smallp.tile([P, P], bf16, tag="dcm")
            nc.vector.tensor_scalar(
                out=dcp[:], in0=ident_t[:], scalar1=carry_sb[:, 0:1],
                scalar2=None, op0=Alu.mult)
            nc.vector.tensor_scalar(
                out=dcm[:], in0=ident_t[:], scalar1=carry_sb[:, 1:2],
                scalar2=None, op0=Alu.mult)

            # bias = Cp*Cm - 2kB (per-partition scalar for the Ln)
            cpcm = smallp.tile([P, 1], f32, tag="cpcm")
            nc.vector.tensor_scalar(
                out=cpcm[:], in0=carry_sb[:, 0:1], scalar1=carry_sb[:, 1:2],
                scalar2=None, op0=Alu.mult)
            bias_t = smallp.tile([P, 1], f32, tag="bias_t")
            nc.vector.tensor_add(bias_t[:], cpcm[:], neg_lbase_t[:])

            # ---- phase B: den = x1 + Cp*sm + Cm*sp + (CpCm - 2kB), log.
            # Carry products on the PE (PSUM-accumulated, one bank per
            # 512-col sub-tile); DVE adds x1; ACT takes the log. ----
            HB = 512
            for c in range(NCH):
                F = FLIST[c]
                o = OFFS[c]
                t2 = workp.tile([P, F], bf16, tag=f"t2{F}")
                for h in range(F // HB):
                    hs = slice(o + h * HB, o + (h + 1) * HB)
                    ps_h = psumbp.tile([P, HB], f32, tag="psB")
                    nc.tensor.matmul(ps_h[:], dcp[:], sm_t[:, hs],
                                     start=True, stop=False)
                    nc.tensor.matmul(ps_h[:], dcm[:], sp_t[:, hs],
                                     start=False, stop=True)
                    nc.vector.tensor_add(t2[:, h * HB:(h + 1) * HB],
                                         x1_t[:, hs], ps_h[:])
                ln_o = workp.tile([P, F], bf16, tag=f"ln{F}")
                nc.scalar.activation(ln_o[:], t2[:], Act.Ln,
                                     bias=bias_t[:],
                                     accum_out=aln[:, c:c + 1])

            rll = smallp.tile([P, 1], f32, tag="rll")
            nc.vector.tensor_reduce(rll[:], aln[:], axis=mybir.AxisListType.X,
                                    op=Alu.add)

            part_ps = psump.tile([1, 1], f32, tag="part")
            nc.tensor.matmul(part_ps[:], ones_col_t[:], rll[:],
                             start=True, stop=True)
            part_sb = smallp.tile([1, 1], f32, tag="part_sb")
            nc.scalar.copy(part_sb[:], part_ps[:])
            nc.sync.dma_start(out_part, part_sb[:])

    nc.compile()
    return nc


def _get_nc():
    if "nc" not in _CACHE:
        _CACHE["nc"] = _build_nc()
    return _CACHE["nc"]


def _make_in_maps(pred, target):
    pred = np.ascontiguousarray(np.asarray(pred, dtype=np.float32))
    target = np.ascontiguousarray(np.asarray(target, dtype=np.float32))
    assert pred.shape == (N,) and target.shape == (N,)

    order = np.argsort(-target, kind="stable")  # matches jnp stable argsort
    sp = pred[order]
    u = sp[H - 1:: -1]  # sp[H-1-t]
    v = sp[H:]          # sp[H+t]

    strict = np.triu(np.ones((P, P), np.float32), 1)  # [q,p]=1 iff q<p
    from ml_dtypes import bfloat16 as _bf
    ident = np.eye(P, dtype=np.float32).astype(_bf)  # 0/1: exact in bf16
    ones_col = np.ones((P, 1), np.float32)

    in_maps = []
    for k in range(NCORES):
        mask = np.zeros((NCORES, P), np.float32)
        mask[:k, :] = 1.0
        in_maps.append({
            "u_in": np.ascontiguousarray(u[k * B:(k + 1) * B].reshape(P, C)),
            "v_in": np.ascontiguousarray(v[k * B:(k + 1) * B].reshape(P, C)),
            "maskbc": mask,
            "strict": strict,
            "ident": ident,
            "ones_col": ones_col,
            "neg_lbase": np.full((P, 1), -2.0 * k * B, np.float32),
        })
    return in_maps, u, v


def _run(in_maps, trace=False):
    from concourse import bass_utils
    return bass_utils.run_bass_kernel_spmd(
        _get_nc(), in_maps, list(range(NCORES)), trace=trace
    )


def _finish(res, u, v):
    partials = [r["partial"].reshape(()) for r in res.results]
    lnsum = np.sum(np.asarray(partials, dtype=np.float64))
    log_num = np.sum(u.astype(np.float64)) - np.sum(v.astype(np.float64))
    loss = np.float32(lnsum - log_num)
    return np.asarray(loss, dtype=np.float32).reshape(())


def kernel(pred, target):
    in_maps, u, v = _make_in_maps(pred, target)
    res = _run(in_maps)
    return _finish(res, u, v)


def kernel_traced(pred, target):
    in_maps, u, v = _make_in_maps(pred, target)
    res = _run(in_maps, trace=True)
    return _finish(res, u, v), res
